# revision 41
# baseline (speedup 1.0000x reference)
"""Trainium2 Bass kernel for a 2-stage 13-organ Dice loss.

Math (all organ weights are 1.0, so the per-organ fold collapses to sums):
  for stage s, batch b:
    num[s,b] = 2 * sum_{c in 1..13} sum_v pred_s[b,c,v] * [target[b,v]==c]
    den[s,b] = sum_{c in 1..13} sum_v pred_s[b,c,v]^2 + count(target[b]!=0) + 13*EPS
  dice[b] = num[1,b]/den[1,b] + num[2,b]/den[2,b]
  loss    = mean_b(2 - dice[b])

Sharding: the 48-slice depth axis is split 6-per-core across 8 NeuronCores;
each core handles both batches, both stages, and organ channels 1..13
(channel 0 is background and never touches the device). Each core emits
per-partition partial sums (a few KB); the host does the final reduction
and dice division.

The kernel streams pred in bf16 (host-side cast). The loss is a ratio of
sums over ~40M elements, so the bf16 quantization noise (~1e-3 relative
per element, zero-mean) averages down to ~1e-6 on the final scalar.

Per-core device program (Tile framework; no PE/PSUM needed):
  - DVE builds the 13 one-hot masks for a whole batch's target in bf16
    with tensor_scalar(is_equal) (4x perf mode), plus a zero-count via a
    fused accum_out.
  - DVE scalar_tensor_tensor fuses (pred * 2) * mask with the
    per-partition numerator sum in one pass.
  - ACT activation(Square) computes squares with a fused per-partition
    accum_out (fp32) for the denominator.
All reductions land in small f32 "slot" tiles that are DMA'd out.
"""

import numpy as np
import ml_dtypes

import concourse.bacc as bacc
import concourse.mybir as mybir
import concourse.tile as tile
from concourse.bass_utils import run_bass_kernel_spmd

N_CORES = 8
S = 2  # stages
B = 2  # batch
C = 13  # organ channels (pred channels 1..13; channel 0 skipped)
D = 48  # depth
D_SH = D // N_CORES  # 6 depth slices per core
HW = 256 * 256  # voxels per (b, d) slab
PJ = HW // 128  # 512 free elems per partition per slab
DG = 1  # depth slices per pred tile (DMA batching)
# Work split across engines (channels out of C=13). The full numerator runs
# on TensorE: each one-hot mask chunk is loaded as the stationary operand
# once and multiplied against BOTH stages' pred chunks in a single N=256
# matmul (stationary reuse halves the LDWEIGHTS bill). The denominator
# squares run on ScalarE for the first NACT_DEN channels and on VectorE
# (scalar_tensor_tensor) for the rest. Chosen so PE / ACT / DVE / DMA all
# land near the same busy time.
NACT_DEN = 9
EPS = 1e-5

F32 = mybir.dt.float32
BF16 = mybir.dt.bfloat16


def build_program(d_sh: int = D_SH, pj: int = PJ) -> bacc.Bacc:
    """Build the per-core SPMD Bass program (bf16 inputs).

    The host pre-packs inputs into the exact SBUF layout so every DMA is a
    fully contiguous block:
      pred [S, B, G, 128, C*DG*pj] bf16 — element [.., p, c*DG*pj + d*pj + j]
        = pred_orig[s, b, organ c+1, depth g*DG+d, voxel p*pj+j]
      tgt  [B, 128, d_sh*pj] bf16      — element [b, p, d*pj + j]

    Outputs (per core):
      onum [128, 128*S*B] f32 — PSUM blocks of the TensorE "diagonal
        trick": cols [(b*S+s)*128, +128) hold M[i,j] = sum_chunks
        sum_p mask_chunk[p,i]*pred_chunk[p,j]; the DIAGONAL sums to
        sum(pred*onehot) for that (s,b). Host extracts the trace.
      oden [128,32] f32 (slot idx = (b*G + g)*S + s; per-partition sum
        of squares of channels [0, NACT_DEN) from the ACT accumulator)
      osl  [128,64] f32 (DVE slots: col 32+slot = sum of squares of
        channels [NACT_DEN, C); cols 0-31 are zero)
      ocnt [128,16] f32 (slot idx = b*G + g; per-partition counts of
        target==0)
    """
    assert d_sh % DG == 0
    w = min(128, DG * pj)  # matmul chunk width (128 at full size)
    assert (DG * pj) % w == 0
    G = d_sh // DG
    K_CHUNKS = (DG * pj) // w
    nc = bacc.Bacc(target_bir_lowering=False)
    pred = nc.dram_tensor(
        "pred", [S, B, G, 128, C * DG * pj], BF16, kind="ExternalInput"
    )
    tgt = nc.dram_tensor("tgt", [B, 128, d_sh * pj], BF16, kind="ExternalInput")
    onum = nc.dram_tensor("onum", [128, 128 * S * B], F32, kind="ExternalOutput")
    oden = nc.dram_tensor("oden", [128, 32], F32, kind="ExternalOutput")
    osl = nc.dram_tensor("osl", [128, 64], F32, kind="ExternalOutput")
    ocnt = nc.dram_tensor("ocnt", [128, 16], F32, kind="ExternalOutput")
    # number of matmuls accumulated into each per-b PSUM block
    mm_total = G * C * K_CHUNKS

    with tile.TileContext(nc) as tc:
        with (
            tc.tile_pool(name="tpool", bufs=2) as tpool,
            tc.tile_pool(name="ppool", bufs=5) as ppool,
            tc.tile_pool(name="mpool", bufs=2) as mpool,
            tc.tile_pool(name="dpool", bufs=1) as dpool,
            tc.tile_pool(name="spool", bufs=1) as spool,
            tc.tile_pool(name="qpool", bufs=1, space="PSUM") as qpool,
        ):
            den_slots = spool.tile([128, 32], F32, tag="den")
            sl_slots = spool.tile([128, 64], F32, tag="sl")
            cnt_slots = spool.tile([128, 16], F32, tag="cnt")
            numsb = spool.tile([128, 128 * S * B], F32, tag="numsb")
            # Unused slot columns are DMA'd out; zero them so outputs are
            # deterministic.
            nc.vector.memset(den_slots[:, :], 0.0)
            nc.vector.memset(sl_slots[:, :], 0.0)
            nc.vector.memset(cnt_slots[:, :], 0.0)
            nc.vector.memset(numsb[:, :], 0.0)
            psums = {
                b: qpool.tile([128, S * 128], F32, tag=f"ps{b}", name=f"psum_{b}")
                for b in range(B)
            }
            mm_count = {k: 0 for k in psums}

            gpj = DG * pj
            for b in range(B):
                tb = tpool.tile([128, d_sh * pj], BF16, tag="tb")
                nc.sync.dma_start(out=tb[:, :], in_=tgt[b])
                for g in range(G):
                    # 13 one-hot masks for this depth-pair's target (bf16
                    # in/out -> 4x DVE mode), matching the pred tile layout.
                    masks = mpool.tile([128, C, gpj], BF16, tag="masks")
                    for c in range(C):
                        nc.vector.tensor_scalar(
                            masks[:, c, :],
                            tb[:, g * gpj : (g + 1) * gpj],
                            float(c + 1),
                            None,
                            mybir.AluOpType.is_equal,
                        )
                    zdummy = dpool.tile([128, gpj], BF16, tag="zd")
                    nc.vector.tensor_scalar(
                        zdummy[:, :],
                        tb[:, g * gpj : (g + 1) * gpj],
                        0.0,
                        None,
                        mybir.AluOpType.is_equal,
                        mybir.AluOpType.add,
                        accum_out=cnt_slots[:, b * G + g : b * G + g + 1],
                    )
                    # One DMA brings BOTH stages' (b,g) pred block.
                    pt = ppool.tile([128, S, C * gpj], BF16, tag="pt")
                    nc.sync.dma_start(
                        out=pt[:, :, :],
                        in_=pred[:, b, g].rearrange("s p f -> p s f"),
                    )
                    for s in range(S):
                        slot = (b * G + g) * S + s
                        # Denominator squares: ScalarE for channels
                        # [0, NACT_DEN), VectorE (STT) for the rest.
                        sdummy = dpool.tile([128, NACT_DEN * gpj], BF16, tag="sd")
                        nc.scalar.activation(
                            sdummy[:, :],
                            pt[:, s, : NACT_DEN * gpj],
                            mybir.ActivationFunctionType.Square,
                            accum_out=den_slots[:, slot : slot + 1],
                        )
                        sdummy2 = dpool.tile(
                            [128, (C - NACT_DEN) * gpj], BF16, tag="sd2"
                        )
                        nc.vector.scalar_tensor_tensor(
                            out=sdummy2[:, :],
                            in0=pt[:, s, NACT_DEN * gpj :],
                            scalar=1.0,
                            in1=pt[:, s, NACT_DEN * gpj :],
                            op0=mybir.AluOpType.mult,
                            op1=mybir.AluOpType.mult,
                            accum_out=sl_slots[:, 32 + slot : 32 + slot + 1],
                        )
                    # Numerator on TensorE: load each mask chunk as the
                    # stationary ONCE and stream both stages' pred chunks
                    # as one N=2*w moving operand; accumulate into the
                    # per-b PSUM block (host extracts the diagonals).
                    ps = psums[b]
                    for c in range(C):
                        for k in range(K_CHUNKS):
                            col = slice(c * gpj + k * w, c * gpj + (k + 1) * w)
                            mm_count[b] += 1
                            nc.tensor.matmul(
                                ps[:w, : S * w],
                                masks[:, c, k * w : (k + 1) * w],
                                pt[:, :, col],
                                start=(mm_count[b] == 1),
                                stop=(mm_count[b] == mm_total),
                            )

            for b in range(B):
                for s in range(S):
                    q = b * S + s
                    nc.vector.tensor_copy(
                        numsb[:w, q * 128 : q * 128 + w],
                        psums[b][:w, s * w : s * w + w],
                    )
            nc.sync.dma_start(out=onum[:, :], in_=numsb[:, :])
            nc.sync.dma_start(out=oden[:, :], in_=den_slots[:, :])
            nc.sync.dma_start(out=osl[:, :], in_=sl_slots[:, :])
            nc.sync.dma_start(out=ocnt[:, :], in_=cnt_slots[:, :])
    nc.finalize()
    return nc


def shard_inputs(pred_stage1, pred_stage2, target, n_cores=N_CORES, d_sh=D_SH):
    """Slice off the background channel, split depth per core, cast to bf16,
    and pack into the device layout (see build_program docstring)."""
    G = d_sh // DG
    in_maps = []
    p1 = np.asarray(pred_stage1)
    p2 = np.asarray(pred_stage2)
    tg = np.asarray(target)
    for k in range(n_cores):
        d0, d1 = k * d_sh, (k + 1) * d_sh
        pshard = np.empty((S, B, G, 128, C * DG * PJ), ml_dtypes.bfloat16)
        for s, src in enumerate((p1, p2)):
            x = src[:, 1:, d0:d1].reshape(B, C, G, DG, 128, PJ)
            x = x.transpose(0, 2, 4, 1, 3, 5)  # (B, G, 128, C, DG, PJ)
            pshard[s] = x.reshape(B, G, 128, C * DG * PJ)
        t = tg[:, d0:d1].reshape(B, d_sh, 128, PJ).transpose(0, 2, 1, 3)
        tshard = t.reshape(B, 128, d_sh * PJ).astype(ml_dtypes.bfloat16)
        in_maps.append({"pred": pshard, "tgt": tshard})
    return in_maps


def combine_results(results, d_sh=D_SH, pj=PJ):
    """Host-side final reduction of the per-core per-partition partials."""
    G = d_sh // DG
    num = np.zeros((S, B), np.float64)
    den = np.zeros((S, B), np.float64)
    cnt = np.zeros((B,), np.float64)
    group_voxels = 128 * pj * DG
    for r in results:
        onum = r["onum"].astype(np.float64)
        oden = r["oden"].astype(np.float64)
        osl = r["osl"].astype(np.float64)
        ocnt = r["ocnt"].astype(np.float64)
        for b in range(B):
            for s in range(S):
                q = b * S + s
                num[s, b] += 2.0 * np.trace(onum[:, q * 128 : (q + 1) * 128])
            for g in range(G):
                cnt[b] += group_voxels - ocnt[:, b * G + g].sum()
                for s in range(S):
                    slot = (b * G + g) * S + s
                    num[s, b] += osl[:, slot].sum()
                    den[s, b] += oden[:, slot].sum() + osl[:, 32 + slot].sum()
    dice = np.zeros(B, np.float64)
    for b in range(B):
        for s in range(S):
            dice[b] += num[s, b] / (den[s, b] + cnt[b] + C * EPS)
    loss = np.mean(2.0 - dice)
    return np.array(loss, dtype=np.float32)


def kernel(pred_stage1, pred_stage2, target):
    in_maps = shard_inputs(pred_stage1, pred_stage2, target)
    nc = build_program()
    # The first multi-core execution of a freshly loaded NEFF occasionally
    # hits a transient NRT_EXEC_UNIT_UNRECOVERABLE; a retry succeeds.
    last_err = None
    for _ in range(3):
        try:
            res = run_bass_kernel_spmd(nc, in_maps, list(range(N_CORES)))
            return combine_results(res.results)
        except Exception as e:  # noqa: BLE001
            last_err = e
    raise last_err


# revision 43
# speedup vs baseline: 1.0545x; 1.0545x over previous
"""Trainium2 Bass kernel for a 2-stage 13-organ Dice loss.

Math (all organ weights are 1.0, so the per-organ fold collapses to sums):
  for stage s, batch b:
    num[s,b] = 2 * sum_{c in 1..13} sum_v pred_s[b,c,v] * [target[b,v]==c]
    den[s,b] = sum_{c in 1..13} sum_v pred_s[b,c,v]^2 + count(target[b]!=0) + 13*EPS
  dice[b] = num[1,b]/den[1,b] + num[2,b]/den[2,b]
  loss    = mean_b(2 - dice[b])

Sharding: the 48-slice depth axis is split 6-per-core across 8 NeuronCores;
each core handles both batches, both stages, and organ channels 1..13
(channel 0 is background and never touches the device). Each core emits
per-partition partial sums (a few KB); the host does the final reduction
and dice division.

The kernel streams pred in bf16 (host-side cast). The loss is a ratio of
sums over ~40M elements, so the bf16 quantization noise (~1e-3 relative
per element, zero-mean) averages down to ~1e-6 on the final scalar.

Per-core device program (Tile framework; no PE/PSUM needed):
  - DVE builds the 13 one-hot masks for a whole batch's target in bf16
    with tensor_scalar(is_equal) (4x perf mode), plus a zero-count via a
    fused accum_out.
  - DVE scalar_tensor_tensor fuses (pred * 2) * mask with the
    per-partition numerator sum in one pass.
  - ACT activation(Square) computes squares with a fused per-partition
    accum_out (fp32) for the denominator.
All reductions land in small f32 "slot" tiles that are DMA'd out.
"""

import numpy as np
import ml_dtypes

import concourse.bacc as bacc
import concourse.mybir as mybir
import concourse.tile as tile
from concourse.bass_utils import run_bass_kernel_spmd

N_CORES = 8
S = 2  # stages
B = 2  # batch
C = 13  # organ channels (pred channels 1..13; channel 0 skipped)
D = 48  # depth
D_SH = D // N_CORES  # 6 depth slices per core
HW = 256 * 256  # voxels per (b, d) slab
PJ = HW // 128  # 512 free elems per partition per slab
DG = 2  # depth slices per pred tile (DMA batching)
# Work split across engines (channels out of C=13). The full numerator runs
# on TensorE: each one-hot mask chunk is loaded as the stationary operand
# once and multiplied against BOTH stages' pred chunks in a single N=256
# matmul (stationary reuse halves the LDWEIGHTS bill). The denominator
# squares run on ScalarE for the first NACT_DEN channels and on VectorE
# (scalar_tensor_tensor) for the rest. Chosen so PE / ACT / DVE / DMA all
# land near the same busy time.
NACT_DEN = 9
EPS = 1e-5

F32 = mybir.dt.float32
BF16 = mybir.dt.bfloat16


def build_program(d_sh: int = D_SH, pj: int = PJ) -> bacc.Bacc:
    """Build the per-core SPMD Bass program (bf16 inputs).

    The host pre-packs inputs into the exact SBUF layout so every DMA is a
    fully contiguous block:
      pred [S, B, G, 128, C*DG*pj] bf16 — element [.., p, c*DG*pj + d*pj + j]
        = pred_orig[s, b, organ c+1, depth g*DG+d, voxel p*pj+j]
      tgt  [B, 128, d_sh*pj] bf16      — element [b, p, d*pj + j]

    Outputs (per core):
      onum [128, 128*S*B] f32 — PSUM blocks of the TensorE "diagonal
        trick": cols [(b*S+s)*128, +128) hold M[i,j] = sum_chunks
        sum_p mask_chunk[p,i]*pred_chunk[p,j]; the DIAGONAL sums to
        sum(pred*onehot) for that (s,b). Host extracts the trace.
      oden [128,32] f32 (slot idx = (b*G + g)*S + s; per-partition sum
        of squares of channels [0, NACT_DEN) from the ACT accumulator)
      osl  [128,64] f32 (DVE slots: col 32+slot = sum of squares of
        channels [NACT_DEN, C); cols 0-31 are zero)
      ocnt [128,16] f32 (slot idx = b*G + g; per-partition counts of
        target==0)
    """
    assert d_sh % DG == 0
    w = min(128, DG * pj)  # matmul chunk width (128 at full size)
    assert (DG * pj) % w == 0
    G = d_sh // DG
    K_CHUNKS = (DG * pj) // w
    nc = bacc.Bacc(target_bir_lowering=False)
    pred = nc.dram_tensor(
        "pred", [S, B, G, 128, C * DG * pj], BF16, kind="ExternalInput"
    )
    tgt = nc.dram_tensor("tgt", [B, 128, d_sh * pj], BF16, kind="ExternalInput")
    onum = nc.dram_tensor("onum", [128, 128 * S * B], F32, kind="ExternalOutput")
    oden = nc.dram_tensor("oden", [128, 32], F32, kind="ExternalOutput")
    osl = nc.dram_tensor("osl", [128, 64], F32, kind="ExternalOutput")
    ocnt = nc.dram_tensor("ocnt", [128, 16], F32, kind="ExternalOutput")
    # number of matmuls accumulated into each per-b PSUM block
    mm_total = G * C * K_CHUNKS

    with tile.TileContext(nc) as tc:
        with (
            tc.tile_pool(name="tpool", bufs=2) as tpool,
            tc.tile_pool(name="ppool", bufs=2) as ppool,
            tc.tile_pool(name="mpool", bufs=2) as mpool,
            tc.tile_pool(name="dpool", bufs=1) as dpool,
            tc.tile_pool(name="spool", bufs=1) as spool,
            tc.tile_pool(name="qpool", bufs=1, space="PSUM") as qpool,
        ):
            den_slots = spool.tile([128, 32], F32, tag="den")
            sl_slots = spool.tile([128, 64], F32, tag="sl")
            cnt_slots = spool.tile([128, 16], F32, tag="cnt")
            numsb = spool.tile([128, 128 * S * B], F32, tag="numsb")
            # Unused slot columns are DMA'd out; zero them so outputs are
            # deterministic.
            nc.vector.memset(den_slots[:, :], 0.0)
            nc.vector.memset(sl_slots[:, :], 0.0)
            nc.vector.memset(cnt_slots[:, :], 0.0)
            nc.vector.memset(numsb[:, :], 0.0)
            psums = {
                b: qpool.tile([128, S * 128], F32, tag=f"ps{b}", name=f"psum_{b}")
                for b in range(B)
            }
            mm_count = {k: 0 for k in psums}

            gpj = DG * pj
            for b in range(B):
                tb = tpool.tile([128, d_sh * pj], BF16, tag="tb")
                nc.sync.dma_start(out=tb[:, :], in_=tgt[b])
                for g in range(G):
                    # 13 one-hot masks for this depth-pair's target (bf16
                    # in/out -> 4x DVE mode), matching the pred tile layout.
                    masks = mpool.tile([128, C, gpj], BF16, tag="masks")
                    for c in range(C):
                        nc.vector.tensor_scalar(
                            masks[:, c, :],
                            tb[:, g * gpj : (g + 1) * gpj],
                            float(c + 1),
                            None,
                            mybir.AluOpType.is_equal,
                        )
                    zdummy = dpool.tile([128, gpj], BF16, tag="zd")
                    nc.vector.tensor_scalar(
                        zdummy[:, :],
                        tb[:, g * gpj : (g + 1) * gpj],
                        0.0,
                        None,
                        mybir.AluOpType.is_equal,
                        mybir.AluOpType.add,
                        accum_out=cnt_slots[:, b * G + g : b * G + g + 1],
                    )
                    # One DMA brings BOTH stages' (b,g) pred block.
                    pt = ppool.tile([128, S, C * gpj], BF16, tag="pt")
                    nc.sync.dma_start(
                        out=pt[:, :, :],
                        in_=pred[:, b, g].rearrange("s p f -> p s f"),
                    )
                    for s in range(S):
                        slot = (b * G + g) * S + s
                        # Denominator squares: ScalarE for channels
                        # [0, NACT_DEN), VectorE (STT) for the rest.
                        sdummy = dpool.tile([128, NACT_DEN * gpj], BF16, tag="sd")
                        nc.scalar.activation(
                            sdummy[:, :],
                            pt[:, s, : NACT_DEN * gpj],
                            mybir.ActivationFunctionType.Square,
                            accum_out=den_slots[:, slot : slot + 1],
                        )
                        sdummy2 = dpool.tile(
                            [128, (C - NACT_DEN) * gpj], BF16, tag="sd2"
                        )
                        nc.vector.scalar_tensor_tensor(
                            out=sdummy2[:, :],
                            in0=pt[:, s, NACT_DEN * gpj :],
                            scalar=1.0,
                            in1=pt[:, s, NACT_DEN * gpj :],
                            op0=mybir.AluOpType.mult,
                            op1=mybir.AluOpType.mult,
                            accum_out=sl_slots[:, 32 + slot : 32 + slot + 1],
                        )
                    # Numerator on TensorE: load each mask chunk as the
                    # stationary ONCE and stream both stages' pred chunks
                    # as one N=2*w moving operand; accumulate into the
                    # per-b PSUM block (host extracts the diagonals).
                    ps = psums[b]
                    for c in range(C):
                        for k in range(K_CHUNKS):
                            col = slice(c * gpj + k * w, c * gpj + (k + 1) * w)
                            mm_count[b] += 1
                            nc.tensor.matmul(
                                ps[:w, : S * w],
                                masks[:, c, k * w : (k + 1) * w],
                                pt[:, :, col],
                                start=(mm_count[b] == 1),
                                stop=(mm_count[b] == mm_total),
                            )

            for b in range(B):
                for s in range(S):
                    q = b * S + s
                    nc.vector.tensor_copy(
                        numsb[:w, q * 128 : q * 128 + w],
                        psums[b][:w, s * w : s * w + w],
                    )
            nc.sync.dma_start(out=onum[:, :], in_=numsb[:, :])
            nc.sync.dma_start(out=oden[:, :], in_=den_slots[:, :])
            nc.sync.dma_start(out=osl[:, :], in_=sl_slots[:, :])
            nc.sync.dma_start(out=ocnt[:, :], in_=cnt_slots[:, :])
    nc.finalize()
    return nc


def shard_inputs(pred_stage1, pred_stage2, target, n_cores=N_CORES, d_sh=D_SH):
    """Slice off the background channel, split depth per core, cast to bf16,
    and pack into the device layout (see build_program docstring)."""
    G = d_sh // DG
    in_maps = []
    p1 = np.asarray(pred_stage1)
    p2 = np.asarray(pred_stage2)
    tg = np.asarray(target)
    for k in range(n_cores):
        d0, d1 = k * d_sh, (k + 1) * d_sh
        pshard = np.empty((S, B, G, 128, C * DG * PJ), ml_dtypes.bfloat16)
        for s, src in enumerate((p1, p2)):
            x = src[:, 1:, d0:d1].reshape(B, C, G, DG, 128, PJ)
            x = x.transpose(0, 2, 4, 1, 3, 5)  # (B, G, 128, C, DG, PJ)
            pshard[s] = x.reshape(B, G, 128, C * DG * PJ)
        t = tg[:, d0:d1].reshape(B, d_sh, 128, PJ).transpose(0, 2, 1, 3)
        tshard = t.reshape(B, 128, d_sh * PJ).astype(ml_dtypes.bfloat16)
        in_maps.append({"pred": pshard, "tgt": tshard})
    return in_maps


def combine_results(results, d_sh=D_SH, pj=PJ):
    """Host-side final reduction of the per-core per-partition partials."""
    G = d_sh // DG
    num = np.zeros((S, B), np.float64)
    den = np.zeros((S, B), np.float64)
    cnt = np.zeros((B,), np.float64)
    group_voxels = 128 * pj * DG
    for r in results:
        onum = r["onum"].astype(np.float64)
        oden = r["oden"].astype(np.float64)
        osl = r["osl"].astype(np.float64)
        ocnt = r["ocnt"].astype(np.float64)
        for b in range(B):
            for s in range(S):
                q = b * S + s
                num[s, b] += 2.0 * np.trace(onum[:, q * 128 : (q + 1) * 128])
            for g in range(G):
                cnt[b] += group_voxels - ocnt[:, b * G + g].sum()
                for s in range(S):
                    slot = (b * G + g) * S + s
                    num[s, b] += osl[:, slot].sum()
                    den[s, b] += oden[:, slot].sum() + osl[:, 32 + slot].sum()
    dice = np.zeros(B, np.float64)
    for b in range(B):
        for s in range(S):
            dice[b] += num[s, b] / (den[s, b] + cnt[b] + C * EPS)
    loss = np.mean(2.0 - dice)
    return np.array(loss, dtype=np.float32)


def kernel(pred_stage1, pred_stage2, target):
    in_maps = shard_inputs(pred_stage1, pred_stage2, target)
    nc = build_program()
    # The first multi-core execution of a freshly loaded NEFF occasionally
    # hits a transient NRT_EXEC_UNIT_UNRECOVERABLE; a retry succeeds.
    last_err = None
    for _ in range(3):
        try:
            res = run_bass_kernel_spmd(nc, in_maps, list(range(N_CORES)))
            return combine_results(res.results)
        except Exception as e:  # noqa: BLE001
            last_err = e
    raise last_err
